# revision 5
# baseline (speedup 1.0000x reference)
"""Trainium2 Bass kernel for the CustomJacobiLayer problem.

Computes out[b,j] = sum_{i,d} P_d(tanh(x[b,i])) * coef[j,i,d]
with P_d the Jacobi(alpha=1,beta=1) polynomials, d=0..7.

Strategy (8 NeuronCores, data-parallel over batch):
  - Each core owns 512 of the 4096 batch rows; coef is replicated.
  - Host-side basis change to monomials: P_d(t) = sum_k C[d,k] t^k, so
        out[b,j] = sum_{i,k} t^k * cc[j,i,k],  cc = coef @ C  (f64 on host).
    The k=0 column is a batch-independent bias added on the host. On
    device only the powers t^2..t^7 are needed: 6 VectorE multiplies
    (fp16 2x mode) instead of the 17-op Jacobi recurrence chain.
  - Device: ScalarE tanh (fp16), 6 VectorE mults, 112 accumulating
    TensorE matmuls (fp16, N=512, K-contiguous) into 4 PSUM banks,
    staged to SBUF as fp16 and DMA'd out (upcast to f32 on the host).
  - Schedule: PE warm-up matmuls begin right at kernel start (gated only
    on a GpSimd memset) so the HAM clock gate releases early; x arrives
    via Sync-engine HWDGE (x0 first, then x1, x23), coef via GpSimd
    SWDGE in consumption order (cf1-ic0, cf1-rest, cf2, cf3, cf45,
    cf67). After the real matmuls a short PE keep-alive burst holds the
    full clock through the framework's fixed ~250-semaphore cleanup
    sweep, which otherwise runs at half clock.

Numerics (vs f64 reference, emulated): max err / max|out| ~6.7e-3 --
fp16 matmul inputs with monomial-basis coefficients, fp32 PSUM
accumulation; gate is 2e-2.
"""

import numpy as np

ORDER = 7
ALPHA = 1.0
BETA = 1.0
B_FULL, I_DIM, O_DIM = 4096, 512, 512
N_CORES = 8
BS = B_FULL // N_CORES  # 512 batch rows per core
P = 128                 # SBUF partitions
IC = I_DIM // P         # 4 i-chunks
BT = BS // P            # 4 batch tiles per core

N_WARM = 6              # PE clock-ramp warm-up matmuls
N_KEEP = 10             # PE keep-alive matmuls after the real stream


def _monomial_matrix():
    """C[d,k] with P_d(t) = sum_k C[d,k] t^k (f64, exact recurrence)."""
    a, b = ALPHA, BETA
    k1 = np.zeros(ORDER + 1)
    k2 = np.zeros(ORDER + 1)
    k3 = np.zeros(ORDER + 1)
    for i in range(2, ORDER + 1):
        k1[i] = (2 * i + a + b) * (2 * i + a + b - 1) / (2 * i * (i + a + b))
        k2[i] = (
            (2 * i + a + b - 1) * (a * a - b * b)
            / (2 * i * (i + a + b) * (2 * i + a + b - 2))
        )
        k3[i] = (
            (i + a - 1) * (i + b - 1) * (2 * i + a + b)
            / (i * (i + a + b) * (2 * i + a + b - 2))
        )
    C = np.zeros((ORDER + 1, ORDER + 1))
    C[0, 0] = 1.0
    C[1, 1] = 0.5 * (a + b + 2.0)
    C[1, 0] = -0.5 * (a - b)
    for d in range(2, ORDER + 1):
        C[d, 1:] += k1[d] * C[d - 1, :-1]
        C[d, :] += k2[d] * C[d - 1, :]
        C[d, :] -= k3[d] * C[d - 2, :]
    return C


_C = _monomial_matrix()

_NC_CACHE = {}


def _build_bass():
    from contextlib import ExitStack
    from concourse import bacc, bass, tile, mybir

    nc = bacc.Bacc(
        "TRN2",
        target_bir_lowering=False,
        debug=False,
        num_devices=1,
    )
    f32 = mybir.dt.float32
    f16 = mybir.dt.float16

    xT = nc.dram_tensor("xT", [I_DIM, BS], f16, kind="ExternalInput")
    cf = nc.dram_tensor("cf", [ORDER, I_DIM, O_DIM], f16, kind="ExternalInput")
    out = nc.dram_tensor("out", [BS, O_DIM], f16, kind="ExternalOutput")

    with tile.TileContext(nc) as tc, ExitStack() as ctx:
        pool = ctx.enter_context(tc.tile_pool(name="main", bufs=1))
        psum = ctx.enter_context(
            tc.tile_pool(name="psum", bufs=1, space=bass.MemorySpace.PSUM)
        )

        # PE warm-up: GpSimd memsets the operand tile at kernel start (the
        # other engines are still in their preambles), so the throwaway
        # matmul burst begins ~6.5us into the trace and the HAM clock gate
        # is mostly released by the time real matmuls issue.
        wtile = pool.tile([P, O_DIM], f16, tag="warm")
        nc.gpsimd.memset(wtile[:], 0.5)
        ps_w = psum.tile([P, O_DIM], f32, tag="ps_w", name="ps_w")
        for w in range(N_WARM):
            nc.tensor.matmul(
                ps_w[:], wtile[:, 0:P], wtile[:],
                start=(w == 0), stop=(w == N_WARM - 1),
            )

        # x in: the HWDGE queues are latency/packet-rate bound (~90-165
        # GB/s per queue), so the four i-chunks are spread over BOTH HWDGE
        # queues -- x0, x1, x3 on Sync (idle engine), x2 on Scalar
        # (enqueued before the auto-inserted activation table load) -- while
        # the high-bandwidth GpSimd SWDGE queue carries coef. tanh runs per
        # chunk on ScalarE as each transfer lands.
        xt = pool.tile([P, IC, BS], f16, tag="x")
        t = pool.tile([P, IC, BS], f16, tag="t")
        nc.scalar.dma_start(xt[:, 2, :], xT[2 * P:3 * P, :])
        nc.sync.dma_start(xt[:, 0, :], xT[0 * P:1 * P, :])
        nc.sync.dma_start(xt[:, 1, :], xT[1 * P:2 * P, :])
        nc.sync.dma_start(xt[:, 3, :], xT[3 * P:4 * P, :])
        for c in range(IC):
            nc.scalar.activation(
                t[:, c, :], xt[:, c, :], mybir.ActivationFunctionType.Tanh
            )

        # coef in via GpSimd SWDGE, enqueued in consumption order with the
        # k=1 ic=0 slice first (it gates the first real matmul group).
        cfa = pool.tile([P, ORDER, IC, O_DIM], f16, tag="cf")
        cf_r = cf.rearrange("d (ic p) j -> p d ic j", p=P)
        nc.gpsimd.dma_start(cfa[:, 0, 0:1, :], cf_r[:, 0, 0:1, :])
        nc.gpsimd.dma_start(cfa[:, 0, 1:4, :], cf_r[:, 0, 1:4, :])
        nc.gpsimd.dma_start(cfa[:, 1, :, :], cf_r[:, 1, :, :])
        nc.gpsimd.dma_start(cfa[:, 2, :, :], cf_r[:, 2, :, :])
        nc.gpsimd.dma_start(cfa[:, 3:5, :, :], cf_r[:, 3:5, :, :])
        nc.gpsimd.dma_start(cfa[:, 5:7, :, :], cf_r[:, 5:7, :, :])

        # Monomial powers of t on VectorE (fp16 tensor_tensor at 2x):
        # p2 = t*t, p3 = p2*t, p4 = p2*p2, p5 = p4*t, p6 = p4*p2,
        # p7 = p4*p3. The chain finishes long before the PE needs it.
        p = [None] * (ORDER + 1)
        p[1] = t
        for k, (ka, kb) in ((2, (1, 1)), (3, (2, 1)), (4, (2, 2)),
                            (5, (4, 1)), (6, (4, 2)), (7, (4, 3))):
            pk = pool.tile([P, IC, BS], f16, tag=f"p{k}")
            nc.vector.tensor_tensor(
                pk[:], p[ka][:], p[kb][:], mybir.AluOpType.mult
            )
            p[k] = pk

        # matmuls: psum[b] += p[k][:, ic, b*128 :+128].T @ cfa[:, k-1, ic, :]
        ps = [
            psum.tile([P, O_DIM], f32, tag=f"ps{b}", name=f"ps{b}")
            for b in range(BT)
        ]
        for k in range(1, ORDER):
            for ic in range(IC):
                first = k == 1 and ic == 0
                for b in range(BT):
                    nc.tensor.matmul(
                        ps[b][:],
                        p[k][:, ic, b * P:(b + 1) * P],
                        cfa[:, k - 1, ic, :],
                        start=first,
                        stop=False,
                    )

        # Final k=7 group runs bank-major so the banks close staggered and
        # three of the four PSUM->SBUF copies + stores hide under the
        # remaining matmuls. The last bank's copy is split across ScalarE
        # and VectorE with two independent output DMAs.
        ot = pool.tile([P, BT, O_DIM], f16, tag="o")
        for b in range(BT):
            for ic in range(IC):
                nc.tensor.matmul(
                    ps[b][:],
                    p[ORDER][:, ic, b * P:(b + 1) * P],
                    cfa[:, ORDER - 1, ic, :],
                    start=False,
                    stop=(ic == IC - 1),
                )
            if b == 0:
                nc.scalar.copy(ot[:, b, :], ps[b][:])
                nc.scalar.dma_start(out[b * P:(b + 1) * P, :], ot[:, b, :])
            elif b == 1:
                nc.vector.tensor_copy(ot[:, b, :], ps[b][:])
                nc.sync.dma_start(out[b * P:(b + 1) * P, :], ot[:, b, :])
            elif b == 2:
                nc.scalar.copy(ot[:, b, :], ps[b][:])
                nc.scalar.dma_start(out[b * P:(b + 1) * P, :], ot[:, b, :])
            else:
                nc.vector.tensor_copy(ot[:, b, 0:O_DIM // 2], ps[b][:, 0:O_DIM // 2])
                nc.sync.dma_start(
                    out[b * P:(b + 1) * P, 0:O_DIM // 2], ot[:, b, 0:O_DIM // 2]
                )
                nc.scalar.copy(ot[:, b, O_DIM // 2:], ps[b][:, O_DIM // 2:])
                nc.scalar.dma_start(
                    out[b * P:(b + 1) * P, O_DIM // 2:], ot[:, b, O_DIM // 2:]
                )

        # PE keep-alive: throwaway matmuls spanning the copy/store window
        # so the HAM clock stays at full rate through the framework's
        # fixed semaphore-cleanup sweep (which runs 2x slower at k=4).
        # The first one reads the staged output tile written by the b=3
        # copy, pinning the burst to the tail (the tile scheduler hoists
        # dependency-free PE work into earlier stream gaps otherwise).
        for w in range(N_KEEP):
            stat = ot[:, 3, 0:P] if w == 0 else wtile[:, 0:P]
            nc.tensor.matmul(
                ps_w[:], stat, wtile[:],
                start=(w == 0), stop=(w == N_KEEP - 1),
            )

    nc.compile()
    return nc


def _get_nc():
    if "nc" not in _NC_CACHE:
        _NC_CACHE["nc"] = _build_bass()
    return _NC_CACHE["nc"]


def _host_prep(x, coef):
    """Shard + transform inputs. Returns (in_maps, bias)."""
    x = np.asarray(x, dtype=np.float32)
    coef = np.asarray(coef, dtype=np.float32)

    # basis change to monomials (f64): cc[j,i,k] = sum_d coef[j,i,d]*C[d,k]
    cc = np.einsum("jid,dk->jik", coef.astype(np.float64), _C)
    # k = 0 column is constant in t -> bias[j]
    bias = cc[:, :, 0].sum(axis=1)  # [O] f64
    cf_dev = np.ascontiguousarray(
        cc[:, :, 1:].transpose(2, 1, 0).astype(np.float16)  # [7, I, O]
    )

    xT = np.ascontiguousarray(x.T.astype(np.float16))  # [I, B] fp16
    in_maps = [
        {"xT": np.ascontiguousarray(xT[:, c * BS:(c + 1) * BS]), "cf": cf_dev}
        for c in range(N_CORES)
    ]
    return in_maps, bias


def kernel(x, coef):
    from concourse.bass_utils import run_bass_kernel_spmd

    nc = _get_nc()
    in_maps, bias = _host_prep(x, coef)
    res = run_bass_kernel_spmd(nc, in_maps, core_ids=list(range(N_CORES)))
    out = np.concatenate(
        [res.results[c]["out"] for c in range(N_CORES)], axis=0
    ).astype(np.float64)
    out += bias[None, :]
    return out.astype(np.float32)


# revision 10
# speedup vs baseline: 1.0112x; 1.0112x over previous
"""Trainium2 Bass kernel for the CustomJacobiLayer problem.

Computes out[b,j] = sum_{i,d} P_d(tanh(x[b,i])) * coef[j,i,d]
with P_d the Jacobi(alpha=1,beta=1) polynomials, d=0..7.

Strategy (8 NeuronCores, data-parallel over batch):
  - Each core owns 512 of the 4096 batch rows; coef is replicated.
  - Host-side basis change to monomials: P_d(t) = sum_k C[d,k] t^k, so
        out[b,j] = sum_{i,k} t^k * cc[j,i,k],  cc = coef @ C  (f64 on host).
    The k=0 column is a batch-independent bias added on the host. On
    device only the powers t^2..t^7 are needed: 6 VectorE multiplies
    (fp16 2x mode) instead of the 17-op Jacobi recurrence chain.
  - Device: ScalarE tanh (fp16), 6 VectorE mults, 112 accumulating
    TensorE matmuls (fp16, N=512, K-contiguous) into 4 PSUM banks,
    staged to SBUF as fp16 and DMA'd out (upcast to f32 on the host).
  - Schedule: PE warm-up matmuls begin right at kernel start (gated only
    on a GpSimd memset) so the HAM clock gate releases early; x arrives
    via Sync-engine HWDGE (x0 first, then x1, x23), coef via GpSimd
    SWDGE in consumption order (cf1-ic0, cf1-rest, cf2, cf3, cf45,
    cf67). After the real matmuls a short PE keep-alive burst holds the
    full clock through the framework's fixed ~250-semaphore cleanup
    sweep, which otherwise runs at half clock.

Numerics (vs f64 reference, emulated): max err / max|out| ~6.7e-3 --
fp16 matmul inputs with monomial-basis coefficients, fp32 PSUM
accumulation; gate is 2e-2.
"""

import numpy as np

ORDER = 7
ALPHA = 1.0
BETA = 1.0
B_FULL, I_DIM, O_DIM = 4096, 512, 512
N_CORES = 8
BS = B_FULL // N_CORES  # 512 batch rows per core
P = 128                 # SBUF partitions
IC = I_DIM // P         # 4 i-chunks
BT = BS // P            # 4 batch tiles per core

N_WARM = 11             # PE clock-ramp warm-up matmuls (N=256 each)
N_KEEP = 4              # floating PE gap-fillers (scheduler slots them
                        # wherever the real stream would stall)


def _monomial_matrix():
    """C[d,k] with P_d(t) = sum_k C[d,k] t^k (f64, exact recurrence)."""
    a, b = ALPHA, BETA
    k1 = np.zeros(ORDER + 1)
    k2 = np.zeros(ORDER + 1)
    k3 = np.zeros(ORDER + 1)
    for i in range(2, ORDER + 1):
        k1[i] = (2 * i + a + b) * (2 * i + a + b - 1) / (2 * i * (i + a + b))
        k2[i] = (
            (2 * i + a + b - 1) * (a * a - b * b)
            / (2 * i * (i + a + b) * (2 * i + a + b - 2))
        )
        k3[i] = (
            (i + a - 1) * (i + b - 1) * (2 * i + a + b)
            / (i * (i + a + b) * (2 * i + a + b - 2))
        )
    C = np.zeros((ORDER + 1, ORDER + 1))
    C[0, 0] = 1.0
    C[1, 1] = 0.5 * (a + b + 2.0)
    C[1, 0] = -0.5 * (a - b)
    for d in range(2, ORDER + 1):
        C[d, 1:] += k1[d] * C[d - 1, :-1]
        C[d, :] += k2[d] * C[d - 1, :]
        C[d, :] -= k3[d] * C[d - 2, :]
    return C


_C = _monomial_matrix()

_NC_CACHE = {}


def _build_bass():
    from contextlib import ExitStack
    from concourse import bacc, bass, tile, mybir

    nc = bacc.Bacc(
        "TRN2",
        target_bir_lowering=False,
        debug=False,
        num_devices=1,
    )
    f32 = mybir.dt.float32
    f16 = mybir.dt.float16

    xT = nc.dram_tensor("xT", [I_DIM, BS], f16, kind="ExternalInput")
    cf = nc.dram_tensor("cf", [ORDER, I_DIM, O_DIM], f16, kind="ExternalInput")
    out = nc.dram_tensor("out", [BS, O_DIM], f16, kind="ExternalOutput")

    with tile.TileContext(nc) as tc, ExitStack() as ctx:
        pool = ctx.enter_context(tc.tile_pool(name="main", bufs=1))
        psum = ctx.enter_context(
            tc.tile_pool(name="psum", bufs=1, space=bass.MemorySpace.PSUM)
        )

        # PE warm-up: GpSimd memsets the operand tile at kernel start (the
        # other engines are still in their preambles), so the throwaway
        # matmul burst begins ~7.3us into the trace and the HAM clock gate
        # is mostly released by the time real matmuls issue. N=256 quanta
        # so the burst ends right as the first tanh chunk lands.
        wtile = pool.tile([P, 2 * P], f16, tag="warm")
        nc.gpsimd.memset(wtile[:], 0.5)
        ps_w = psum.tile([P, 2 * P], f32, tag="ps_w", name="ps_w")
        for w in range(N_WARM):
            nc.tensor.matmul(
                ps_w[:], wtile[:, 0:P], wtile[:],
                start=(w == 0), stop=(w == N_WARM - 1),
            )

        # x in: the HWDGE queues are latency/packet-rate bound (~90-165
        # GB/s per queue), so the four i-chunks are spread over BOTH HWDGE
        # queues -- x0 (split in two for an earlier first tanh), x1, x3 on
        # Sync (idle engine), x2 on Scalar (enqueued before the
        # auto-inserted activation table load) -- while the high-bandwidth
        # GpSimd SWDGE queue carries coef. tanh runs per chunk on ScalarE
        # as each transfer lands.
        xt = pool.tile([P, IC, BS], f16, tag="x")
        t = pool.tile([P, IC, BS], f16, tag="t")
        H = BS // 2
        nc.scalar.dma_start(xt[:, 2, :], xT[2 * P:3 * P, :])
        nc.sync.dma_start(xt[:, 0, 0:H], xT[0 * P:1 * P, 0:H])
        nc.sync.dma_start(xt[:, 0, H:], xT[0 * P:1 * P, H:])
        nc.sync.dma_start(xt[:, 1, :], xT[1 * P:2 * P, :])
        nc.sync.dma_start(xt[:, 3, :], xT[3 * P:4 * P, :])
        nc.scalar.activation(
            t[:, 0, 0:H], xt[:, 0, 0:H], mybir.ActivationFunctionType.Tanh
        )
        nc.scalar.activation(
            t[:, 0, H:], xt[:, 0, H:], mybir.ActivationFunctionType.Tanh
        )
        for c in range(1, IC):
            nc.scalar.activation(
                t[:, c, :], xt[:, c, :], mybir.ActivationFunctionType.Tanh
            )

        # coef in via GpSimd SWDGE, enqueued in consumption order with the
        # k=1 ic=0 slice first (it gates the first real matmul group).
        cfa = pool.tile([P, ORDER, IC, O_DIM], f16, tag="cf")
        cf_r = cf.rearrange("d (ic p) j -> p d ic j", p=P)
        nc.gpsimd.dma_start(cfa[:, 0, 0:1, :], cf_r[:, 0, 0:1, :])
        nc.gpsimd.dma_start(cfa[:, 0, 1:4, :], cf_r[:, 0, 1:4, :])
        nc.gpsimd.dma_start(cfa[:, 1, :, :], cf_r[:, 1, :, :])
        nc.gpsimd.dma_start(cfa[:, 2, :, :], cf_r[:, 2, :, :])
        nc.gpsimd.dma_start(cfa[:, 3:5, :, :], cf_r[:, 3:5, :, :])
        nc.gpsimd.dma_start(cfa[:, 5:7, :, :], cf_r[:, 5:7, :, :])

        # Monomial powers of t on VectorE (fp16 tensor_tensor at 2x):
        # p2 = t*t (split in two i-chunk halves so the first half starts
        # right after tanh1), p3 = p2*t, p4 = p2*p2, p5 = p4*t,
        # p6 = p4*p2, p7 = p4*p3. The chain stays ahead of the PE.
        p = [None] * (ORDER + 1)
        p[1] = t
        p2 = pool.tile([P, IC, BS], f16, tag="p2")
        nc.vector.tensor_tensor(
            p2[:, 0:2, :], t[:, 0:2, :], t[:, 0:2, :], mybir.AluOpType.mult
        )
        nc.vector.tensor_tensor(
            p2[:, 2:4, :], t[:, 2:4, :], t[:, 2:4, :], mybir.AluOpType.mult
        )
        p[2] = p2
        for k, (ka, kb) in ((3, (2, 1)), (4, (2, 2)),
                            (5, (4, 1)), (6, (4, 2)), (7, (4, 3))):
            pk = pool.tile([P, IC, BS], f16, tag=f"p{k}")
            nc.vector.tensor_tensor(
                pk[:], p[ka][:], p[kb][:], mybir.AluOpType.mult
            )
            p[k] = pk

        # matmuls: psum[b] += p[k][:, ic, b*128 :+128].T @ cfa[:, k-1, ic, :]
        ps = [
            psum.tile([P, O_DIM], f32, tag=f"ps{b}", name=f"ps{b}")
            for b in range(BT)
        ]
        for k in range(1, ORDER):
            for ic in range(IC):
                first = k == 1 and ic == 0
                for b in range(BT):
                    nc.tensor.matmul(
                        ps[b][:],
                        p[k][:, ic, b * P:(b + 1) * P],
                        cfa[:, k - 1, ic, :],
                        start=first,
                        stop=False,
                    )

        # Final k=7 group runs bank-major so the banks close staggered and
        # most PSUM->SBUF copies + stores hide under the remaining
        # matmuls. The exec time is gated by the LAST output DMA's
        # completion (the framework tail after it is fixed), so the last
        # bank's copy+store chain is split across VectorE+Sync and
        # GpSimd (both idle by then) to shorten it.
        ot = pool.tile([P, BT, O_DIM], f16, tag="o")
        Ho = O_DIM // 2
        for b in range(BT):
            for ic in range(IC):
                nc.tensor.matmul(
                    ps[b][:],
                    p[ORDER][:, ic, b * P:(b + 1) * P],
                    cfa[:, ORDER - 1, ic, :],
                    start=False,
                    stop=(ic == IC - 1),
                )
            if b == 0:
                nc.scalar.copy(ot[:, b, :], ps[b][:])
                nc.scalar.dma_start(out[b * P:(b + 1) * P, :], ot[:, b, :])
            elif b == 1:
                nc.vector.tensor_copy(ot[:, b, :], ps[b][:])
                nc.sync.dma_start(out[b * P:(b + 1) * P, :], ot[:, b, :])
            elif b == 2:
                nc.scalar.copy(ot[:, b, :], ps[b][:])
                nc.scalar.dma_start(out[b * P:(b + 1) * P, :], ot[:, b, :])
            else:
                nc.vector.tensor_copy(ot[:, b, 0:Ho], ps[b][:, 0:Ho])
                nc.sync.dma_start(
                    out[b * P:(b + 1) * P, 0:Ho], ot[:, b, 0:Ho]
                )
                nc.vector.tensor_copy(ot[:, b, Ho:], ps[b][:, Ho:])
                nc.gpsimd.dma_start(
                    out[b * P:(b + 1) * P, Ho:], ot[:, b, Ho:]
                )

        # Floating PE gap-fillers: dependency-free throwaway matmuls the
        # tile scheduler slots wherever the real stream would stall, so
        # PE activity stays continuous and the HAM full-clock grant
        # (~4.2us of uninterrupted PE work) lands as early as possible.
        for w in range(N_KEEP):
            nc.tensor.matmul(
                ps_w[:], wtile[:, 0:P], wtile[:],
                start=(w == 0), stop=(w == N_KEEP - 1),
            )

    nc.compile()
    return nc


def _get_nc():
    if "nc" not in _NC_CACHE:
        _NC_CACHE["nc"] = _build_bass()
    return _NC_CACHE["nc"]


def _host_prep(x, coef):
    """Shard + transform inputs. Returns (in_maps, bias)."""
    x = np.asarray(x, dtype=np.float32)
    coef = np.asarray(coef, dtype=np.float32)

    # basis change to monomials (f64): cc[j,i,k] = sum_d coef[j,i,d]*C[d,k]
    cc = np.einsum("jid,dk->jik", coef.astype(np.float64), _C)
    # k = 0 column is constant in t -> bias[j]
    bias = cc[:, :, 0].sum(axis=1)  # [O] f64
    cf_dev = np.ascontiguousarray(
        cc[:, :, 1:].transpose(2, 1, 0).astype(np.float16)  # [7, I, O]
    )

    xT = np.ascontiguousarray(x.T.astype(np.float16))  # [I, B] fp16
    in_maps = [
        {"xT": np.ascontiguousarray(xT[:, c * BS:(c + 1) * BS]), "cf": cf_dev}
        for c in range(N_CORES)
    ]
    return in_maps, bias


def kernel(x, coef):
    from concourse.bass_utils import run_bass_kernel_spmd

    nc = _get_nc()
    in_maps, bias = _host_prep(x, coef)
    res = run_bass_kernel_spmd(nc, in_maps, core_ids=list(range(N_CORES)))
    out = np.concatenate(
        [res.results[c]["out"] for c in range(N_CORES)], axis=0
    ).astype(np.float64)
    out += bias[None, :]
    return out.astype(np.float32)
